# revision 13
# baseline (speedup 1.0000x reference)
"""Causal self-attention (B=4, S=2048, E=1024, H=16) on 8 trn2 NeuronCores.

Sharding: data parallel over batch (4) x tensor parallel over head groups (2).
Core c handles batch c//2, heads [ (c%2)*8, (c%2)*8+8 ).  Each core computes
its group's QKV projections, causal attention, and a partial output
projection; the host sums the two group partials per batch and adds bo.

Datapath: fp32r (relaxed fp32, 1 moving column/cycle) everywhere except
QT/KT, which are stored fp16.  The PE's moving-operand SBUF port is ~4
bytes/partition/cycle shared across concurrent matmuls, so the head-pair
score matmuls (row groups 0-63 / 64-127, issued back-to-back) stream both
fp16 operands at full rate instead of halving each other.  Scales fold as
Q/64, K/8, so PSUM scores arrive pre-divided by 64 for the exp paths.

exp is split across two engines to un-bottleneck the attention phase:
  - ScalarE: activation(Exp, scale=64) — all chain-critical (single-qb)
    tiles plus most others.
  - VectorE: custom-DVE pair EXPPOLY (deg-3 Taylor of e^y) then SQ6 (six
    squarings), exp(64 s) = poly(s)^64 at ~1e-4 rel err, on a rotating
    subset of double-qb tiles where extra latency hides.
Diagonal k-tiles skip the fully-masked left columns everywhere: exp, the
GpSimd triangle multiply, and the AV matmul all operate on the live column
range only (no zeroing pass).

The attention loop is q-block-pair-major (qp outer, head-pair inner), so
once qp=0 finishes every pair, the first half of the output projection is
interleaved into the qp=1 instruction stream: its PSUM tiles share the
attention accumulator tag and its PSUM->SBUF drains run on GpSimd, filling
PE gaps left by the score->exp->score dependency chain.

The softmax denominator comes from a 65th ones-column in the AV stationary
operand; the raw PSUM row is DMA-bounced through DRAM to broadcast it to 64
partitions, then inverted with the single-op custom-DVE fast reciprocal.
"""

import numpy as np

import concourse.mybir as mybir
import concourse.tile as tile
from concourse import bacc
from concourse.bass_utils import run_bass_kernel_spmd

F32 = mybir.dt.float32
F32R = mybir.dt.float32r
F16 = mybir.dt.float16
Exp = mybir.ActivationFunctionType.Exp
MULT = mybir.AluOpType.mult
ADD = mybir.AluOpType.add

B, S, E, H = 4, 2048, 1024, 16
D = 64          # head dim
HG = 8          # heads per core
G = 512         # group feature width
P = 128
NKT = S // P    # 16 k-tiles
NST = S // P    # 16 s-tiles
QB = 512        # q-block width
NQB = S // QB   # 4
ESUB = E // P   # 8
VW = D + 1      # V stationary width (64 dims + ones column)

_CACHE = {}


def _register_dve_ops():
    """exp(64*y) = (1 + y + y^2/2 + y^3/6)^64 as two custom-DVE ops."""
    if "dve" in _CACHE:
        return _CACHE["dve"]
    from concourse.dve_spec import Spec, Src0, C0, C1, One, sq, lower
    import concourse.dve_ops as dops

    def mk(name, spec):
        for op in dops.OPS:
            if op.name == name:
                return op
        sha = {}
        for ver in ("v3", "v4"):
            s = dops.DveOpSpec(name=name, opcode=0, uops=lower(spec, ver=ver),
                               rd1_en=False)
            sha[ver] = s.sha(ver)
        op = dops.DveOp(name, spec, subdim=False, uops_sha=sha)
        dops.OPS.append(op)
        dops.CUSTOM_DVE_SPECS[name] = spec
        dops._SUB_OPCODE_FOR_NAME[name] = (
            dops._CUSTOM_DVE_ROW_BASE + len(dops.OPS) - 1
        )
        return op

    exppoly = mk("EXPPOLY_ANT", Spec(
        body=((Src0 * C0 + C1) * Src0 + One) * Src0 + One,
        reference=lambda in0, s0, s1: ((in0 * s0 + s1) * in0 + 1) * in0 + 1,
    ))
    sq6 = mk("SQ6_ANT", Spec(
        body=sq(sq(sq(sq(sq(sq(Src0)))))),
        reference=lambda in0: in0 ** 64,
    ))
    _CACHE["dve"] = (exppoly, sq6)
    return exppoly, sq6


def _build_program():
    EXPPOLY, SQ6 = _register_dve_ops()
    nc = bacc.Bacc("TRN2", target_bir_lowering=False, debug=False)

    xt_d = nc.dram_tensor("xt", [E, S], F32R, kind="ExternalInput").ap()
    wq_d = nc.dram_tensor("wq", [E, G], F32R, kind="ExternalInput").ap()
    wk_d = nc.dram_tensor("wk", [E, G], F32R, kind="ExternalInput").ap()
    wv_d = nc.dram_tensor("wv", [E, G], F32R, kind="ExternalInput").ap()
    wo_d = nc.dram_tensor("wo", [G, E], F32R, kind="ExternalInput").ap()
    bq_d = nc.dram_tensor("bq", [P, 4], F32, kind="ExternalInput").ap()
    bk_d = nc.dram_tensor("bk", [P, 4], F32, kind="ExternalInput").ap()
    bv_d = nc.dram_tensor("bv", [P, G], F32, kind="ExternalInput").ap()
    tri_d = nc.dram_tensor("tri", [P, P], F32R, kind="ExternalInput").ap()
    one_d = nc.dram_tensor("one", [P, D], F32R, kind="ExternalInput").ap()
    out_d = nc.dram_tensor("out", [S, E], F32, kind="ExternalOutput").ap()
    # scratch for the reciprocal-row broadcast bounce
    rc_d = nc.dram_tensor("rc_scratch", [HG, NQB, QB], F32, kind="Internal").ap()

    qt_sb = nc.alloc_sbuf_tensor("qt_sb", [P, 4, S], F16).ap()
    kt_sb = nc.alloc_sbuf_tensor("kt_sb", [P, 4, S], F16).ap()
    vx_sb = nc.alloc_sbuf_tensor("vx_sb", [P, NKT, HG, VW], F32R).ap()
    tri_sb = nc.alloc_sbuf_tensor("tri_sb", [P, P], F32R).ap()
    ones_sb = nc.alloc_sbuf_tensor("ones_sb", [P, D], F32R).ap()
    bq_sb = nc.alloc_sbuf_tensor("bq_sb", [P, 4], F32).ap()
    bk_sb = nc.alloc_sbuf_tensor("bk_sb", [P, 4], F32).ap()
    bv_sb = nc.alloc_sbuf_tensor("bv_sb", [P, G], F32).ap()

    with tile.TileContext(nc) as tc:
        xt_r = xt_d.rearrange("(o p) s -> p o s", p=P)

        # ---- QKV projections (xT streamed per 512-token chunk) ----
        with (
            tc.tile_pool(name="w_pool", bufs=1) as wp,
            tc.tile_pool(name="xt_pool", bufs=2) as xp,
            tc.tile_pool(name="proj_ps", bufs=2, space="PSUM") as pp,
        ):
            wqp = wp.tile([P, ESUB, G], F32R, tag="wq", name="wqp")
            wkp = wp.tile([P, ESUB, G], F32R, tag="wk", name="wkp")
            wvp = wp.tile([P, ESUB, G], F32R, tag="wv", name="wvp")
            # first xT chunk loads ahead of the weights so the PE starts
            # sooner; wq fully precedes wk/wv so Q matmuls start first
            xtp0 = xp.tile([P, ESUB, QB], F32R, tag="xt", name="xtp")
            for e in range(ESUB):
                nc.sync.dma_start(
                    wqp[:, e, :], wq_d.rearrange("(o p) f -> p o f", p=P)[:, e, :]
                )
                nc.sync.dma_start(xtp0[:, e, :], xt_r[:, e, 0:QB])
            nc.sync.dma_start(bq_sb[:], bq_d[:])
            for e in range(ESUB):
                nc.sync.dma_start(
                    wkp[:, e, :], wk_d.rearrange("(o p) f -> p o f", p=P)[:, e, :]
                )
            nc.sync.dma_start(bk_sb[:], bk_d[:])
            for e in range(ESUB):
                nc.sync.dma_start(
                    wvp[:, e, :], wv_d.rearrange("(o p) f -> p o f", p=P)[:, e, :]
                )
            nc.sync.dma_start(bv_sb[:], bv_d[:])
            nc.sync.dma_start(tri_sb[:], tri_d[:])
            nc.sync.dma_start(ones_sb[:], one_d[:])
            for qb in range(NQB):
                if qb == 0:
                    xtp = xtp0
                else:
                    xtp = xp.tile([P, ESUB, QB], F32R, tag="xt", name="xtp")
                    nc.sync.dma_start(xtp[:], xt_r[:, :, qb * QB : (qb + 1) * QB])
                for cc in range(4):
                    q_ps = pp.tile([P, QB], F32, tag="q", name="q_ps")
                    for e in range(ESUB):
                        nc.tensor.matmul(
                            q_ps[:],
                            lhsT=wqp[:, e, cc * P : (cc + 1) * P],
                            rhs=xtp[:, e, :],
                            start=(e == 0),
                            stop=(e == ESUB - 1),
                        )
                    # fold bias and the 1/64 share of the softmax scale into Q
                    # (stored fp16; K carries the other 1/8)
                    nc.vector.tensor_scalar(
                        qt_sb[:, cc, qb * QB : (qb + 1) * QB],
                        q_ps[:],
                        bq_sb[:, cc : cc + 1],
                        1.0 / 64.0,
                        ADD,
                        MULT,
                    )
                    k_ps = pp.tile([P, QB], F32, tag="k", name="k_ps")
                    for e in range(ESUB):
                        nc.tensor.matmul(
                            k_ps[:],
                            lhsT=wkp[:, e, cc * P : (cc + 1) * P],
                            rhs=xtp[:, e, :],
                            start=(e == 0),
                            stop=(e == ESUB - 1),
                        )
                    nc.vector.tensor_scalar(
                        kt_sb[:, cc, qb * QB : (qb + 1) * QB],
                        k_ps[:],
                        bk_sb[:, cc : cc + 1],
                        1.0 / 8.0,
                        ADD,
                        MULT,
                    )
                for stl in range(4):
                    st = qb * 4 + stl
                    v_ps = pp.tile([P, G], F32, tag="v", name="v_ps")
                    for e in range(ESUB):
                        nc.tensor.matmul(
                            v_ps[:],
                            lhsT=xtp[:, e, stl * P : (stl + 1) * P],
                            rhs=wvp[:, e, :],
                            start=(e == 0),
                            stop=(e == ESUB - 1),
                        )
                    nc.vector.tensor_tensor(
                        vx_sb[:, st, :, 0:D],
                        v_ps.rearrange("p (h d) -> p h d", d=D),
                        bv_sb.rearrange("p (h d) -> p h d", d=D),
                        ADD,
                    )
                    nc.vector.tensor_copy(
                        vx_sb[:, st, :, D : D + 1],
                        ones_sb[:, 0:HG].rearrange("p (h u) -> p h u", u=1),
                    )

        # ---- attention + output projection ----
        with tc.tile_pool(name="at_pool", bufs=1) as atp:
            at_t = atp.tile([P, 4, S], F32R, name="at_t")
            # wo lives in the same long-lived pool and loads during attention
            wop = atp.tile([P, 4, E], F32R, name="wop")
            nc.sync.dma_start(wop[:], wo_d.rearrange("(o p) n -> p o n", p=P))
            with (
                tc.tile_pool(name="attn_ps", bufs=1, space="PSUM") as ap,
                tc.tile_pool(name="attn_sb", bufs=2) as sp,
            ):
                # heads run in even/odd pairs: the pair's score matmuls use
                # partition bases 0/64 (distinct PE row groups -> concurrent).
                # AV matmuls lag the scores by AV_LAG k-tiles so the PE never
                # waits on exp/mask; accumulators are per-(head, q-block) so
                # normalization runs mid-pass and frees PSUM slots early.
                AV_LAG = 4
                dbl_ctr = [0]

                def emit_outproj(st, n, drain):
                    o_ps = ap.tile([P, QB], F32, tag="a", name="o_ps", bufs=4)
                    for t in range(4):
                        nc.tensor.matmul(
                            o_ps[:],
                            lhsT=at_t[:, t, st * P : (st + 1) * P],
                            rhs=wop[:, t, n * QB : (n + 1) * QB],
                            start=(t == 0),
                            stop=(t == 3),
                        )
                    o_sb = sp.tile([P, QB], F32, tag="ost", name="o_sb", bufs=3)
                    if drain == "vector":
                        nc.vector.tensor_copy(o_sb[:], o_ps[:])
                    else:
                        nc.scalar.copy(o_sb[:], o_ps[:])
                    nc.sync.dma_start(
                        out_d[st * P : (st + 1) * P, n * QB : (n + 1) * QB],
                        o_sb[:],
                    )

                # output projection tiles st<8 interleave into the qb=2/3
                # rounds, st 8-11 into qb=3 (inputs ready block-granularity;
                # emitted only once their at_t inputs are certainly written,
                # so they never head-of-line-block the in-order PE queue)
                feed_a = [(st, n) for st in range(8) for n in range(2)]
                feed_b = [(st, n) for st in range(8, 12) for n in range(2)]

                norm_done = [0, 0, 0, 0]

                def normalize(ctx, h, qb):
                    a_t, heads, sub = ctx
                    norm_done[qb] += 1
                    hb = (h % 2) * D
                    a_ps = a_t[h]
                    # broadcast the RAW denominator row to 64 partitions via a
                    # DRAM bounce (a DRAM source AP may repeat along
                    # partitions, SBUF cannot), then take the reciprocal at
                    # partition base 0 with the fast custom-DVE op (~3e-6 rel
                    # err; it mishandles nonzero partition bases, hence this
                    # order)
                    dn = sp.tile([VW, QB], F32, tag="lg", name="dn", bufs=2)
                    nc.vector.tensor_copy(dn[D:VW, :], a_ps[D:VW, :])
                    nc.sync.dma_start(rc_d[h, qb : qb + 1, :], dn[D:VW, :])
                    db = sp.tile([D, QB], F32, tag="rs", name="db", bufs=2)
                    nc.sync.dma_start(
                        db[:], rc_d[h, qb : qb + 1, :].to_broadcast([D, QB])
                    )
                    rb_sb = sp.tile([D, QB], F32, tag="rbs", name="rb_sb", bufs=2)
                    nc.vector.reciprocal_approx_fast(rb_sb[:], db[:])
                    at_slice = at_t[hb : hb + D, sub, qb * QB : (qb + 1) * QB]
                    if hb == 0:
                        nc.vector.tensor_tensor(at_slice, a_ps[0:D, :], rb_sb[:], MULT)
                    else:
                        tmp = sp.tile([D, QB], F32R, tag="tmp", name="tmp", bufs=2)
                        nc.vector.tensor_tensor(tmp[:], a_ps[0:D, :], rb_sb[:], MULT)
                        nc.sync.dma_start(at_slice, tmp[:])

                def av_main(ctx, pt, kt, qb):
                    # AV over the columns with no triangle-mask dependency:
                    # everything right of the diagonal 128-block (diag k-tile)
                    # or the whole live range (plain k-tile)
                    a_t, heads, sub = ctx
                    if kt // 4 == qb:
                        off = P * (kt % 4 + 1)
                    else:
                        off = 0
                    if off >= QB:
                        return
                    for h in heads:
                        nc.tensor.matmul(
                            a_t[h][0:VW, off:],
                            lhsT=vx_sb[:, kt, h, :],
                            rhs=pt[:, h % 2, off:],
                            start=(kt == 0),
                            stop=False,
                        )

                def av_tri(ctx, pt, kt, qb):
                    # the diagonal 128-column block, gated on the GpSimd
                    # triangle multiply; lags further so the in-order PE queue
                    # never stalls on it.  Carries start (kt==0, runs first
                    # into a fresh bank only for qb==0) and stop/normalize on
                    # the final k-tile.
                    a_t, heads, sub = ctx
                    m = kt % 4
                    last = kt == 4 * qb + 3
                    for h in heads:
                        nc.tensor.matmul(
                            a_t[h][0:VW, P * m : P * (m + 1)],
                            lhsT=vx_sb[:, kt, h, :],
                            rhs=pt[:, h % 2, P * m : P * (m + 1)],
                            start=False,
                            stop=last,
                        )
                    if last:
                        for h in heads:
                            normalize(ctx, h, qb)

                # one software pipeline across all (qb, pair) blocks: the AV/
                # normalize drain of each block interleaves with the next
                # block's score matmuls instead of stalling the in-order PE
                AV_LAG_TRI = 7
                pend_main = []   # (ctx, pt, kt, qb)
                pend_tri = []    # (ctx, pt, kt, qb)
                step = [0]

                def pump():
                    if len(pend_main) > AV_LAG:
                        av_main(*pend_main.pop(0))
                    if len(pend_tri) > AV_LAG_TRI:
                        av_tri(*pend_tri.pop(0))

                for qb in range(NQB):
                    rstep = 0
                    for pair in range(4):
                        heads = (2 * pair, 2 * pair + 1)
                        sub = pair
                        a_t = {
                            h: ap.tile([P, QB], F32, tag="a", name="a_ps", bufs=4)
                            for h in heads
                        }
                        ctx = (a_t, heads, sub)
                        for kt in range(4 * qb + 4):
                            s_t = ap.tile(
                                [P, 2, QB], F32, tag="s", name="s_ps", bufs=2
                            )
                            for h in heads:
                                hb = (h % 2) * D
                                nc.tensor.matmul(
                                    s_t[:, h % 2, :],
                                    lhsT=kt_sb[
                                        hb : hb + D, sub, kt * P : (kt + 1) * P
                                    ],
                                    rhs=qt_sb[
                                        hb : hb + D, sub, qb * QB : (qb + 1) * QB
                                    ],
                                    start=True,
                                    stop=True,
                                )
                            pt = sp.tile(
                                [P, 2, QB], F32R, tag="pt", name="pt", bufs=10
                            )
                            diag = kt // 4 == qb
                            m = kt % 4 if diag else 0
                            # exp engine split: rotate a fixed share of tiles
                            # onto VectorE (two-op poly^64), rest on ScalarE
                            step[0] += 1
                            if step[0] % 5 == 2 and not diag:
                                tx = sp.tile(
                                    [P, 2, QB], F32, tag="tx", name="tx", bufs=2
                                )
                                nc.vector._custom_dve(
                                    EXPPOLY,
                                    out=tx[:, :, P * m :],
                                    in0=s_t[:, :, P * m :],
                                    s0=1.0 / 6.0,
                                    s1=0.5,
                                )
                                nc.vector._custom_dve(
                                    SQ6,
                                    out=pt[:, :, P * m :],
                                    in0=tx[:, :, P * m :],
                                )
                            else:
                                nc.scalar.activation(
                                    pt[:, :, P * m :], s_t[:, :, P * m :],
                                    Exp, scale=64.0,
                                )
                            if diag:
                                for h in heads:
                                    nc.gpsimd.tensor_tensor(
                                        pt[:, h % 2, P * m : P * (m + 1)],
                                        pt[:, h % 2, P * m : P * (m + 1)],
                                        tri_sb[:],
                                        MULT,
                                    )
                                pend_tri.append((ctx, pt, kt, qb))
                            pend_main.append((ctx, pt, kt, qb))
                            pump()
                            rstep += 1
                            if (
                                qb >= 2
                                and feed_a
                                and rstep % 3 == 0
                                and norm_done[feed_a[0][0] // 4] == 8
                            ):
                                emit_outproj(*feed_a.pop(0), drain="vector")
                            elif (
                                qb == 3
                                and feed_b
                                and rstep % 4 == 2
                                and norm_done[feed_b[0][0] // 4] == 8
                            ):
                                emit_outproj(*feed_b.pop(0), drain="vector")
                while pend_main or pend_tri:
                    if pend_main:
                        av_main(*pend_main.pop(0))
                    if pend_tri and (
                        not pend_main
                        or len(pend_tri) > AV_LAG_TRI - AV_LAG
                    ):
                        av_tri(*pend_tri.pop(0))
                for st_n in feed_a + feed_b:
                    emit_outproj(*st_n, drain="scalar")

            # ---- second-half output projection (first half ran inside the
            # attention loop; host adds the other group's partial + bo) ----
            with (
                tc.tile_pool(name="op_ps", bufs=2, space="PSUM") as op,
                tc.tile_pool(name="op_sb", bufs=3) as osp,
            ):
                for st in range(12, NST):
                    for n in range(2):
                        o_ps = op.tile([P, QB], F32, tag="o", name="o_ps")
                        for t in range(4):
                            nc.tensor.matmul(
                                o_ps[:],
                                lhsT=at_t[:, t, st * P : (st + 1) * P],
                                rhs=wop[:, t, n * QB : (n + 1) * QB],
                                start=(t == 0),
                                stop=(t == 3),
                            )
                        o_sb = osp.tile([P, QB], F32, tag="ost", name="o_sb")
                        nc.scalar.copy(o_sb[:], o_ps[:])
                        nc.sync.dma_start(
                            out_d[st * P : (st + 1) * P, n * QB : (n + 1) * QB],
                            o_sb[:],
                        )

    nc.compile()
    return nc


def _prep_inputs(x, Wqkv, bqkv, Wo, bo):
    x = np.asarray(x, np.float32)
    Wqkv = np.asarray(Wqkv, np.float32)
    bqkv = np.asarray(bqkv, np.float32)
    Wo = np.asarray(Wo, np.float32)

    # 128x128 inclusive lower-triangle-in-(q,k) == kl <= ql in [k, q] layout
    kl = np.arange(P)[:, None]
    tri = (kl <= np.arange(P)[None, :]).astype(np.float32)


    in_maps = []
    for c in range(8):
        b, g = divmod(c, 2)
        lo, hi = g * G, (g + 1) * G
        in_maps.append(
            {
                "xt": np.ascontiguousarray(x[b].T),
                "wq": np.ascontiguousarray(Wqkv[:, lo:hi]),
                "wk": np.ascontiguousarray(Wqkv[:, E + lo : E + hi]),
                "wv": np.ascontiguousarray(Wqkv[:, 2 * E + lo : 2 * E + hi]),
                "wo": np.ascontiguousarray(Wo[lo:hi, :]),
                "bq": np.ascontiguousarray(bqkv[lo:hi].reshape(4, P).T),
                "bk": np.ascontiguousarray(bqkv[E + lo : E + hi].reshape(4, P).T),
                "bv": np.tile(bqkv[2 * E + lo : 2 * E + hi][None, :], (P, 1)).astype(
                    np.float32
                ),
                "tri": tri,
                "one": np.ones((P, D), np.float32),
            }
        )
    return in_maps


def kernel(x, Wqkv, bqkv, Wo, bo, _trace=False):
    if "nc" not in _CACHE:
        _CACHE["nc"] = _build_program()
    nc = _CACHE["nc"]

    in_maps = _prep_inputs(x, Wqkv, bqkv, Wo, bo)
    res = run_bass_kernel_spmd(nc, in_maps, core_ids=list(range(8)), trace=_trace)
    _CACHE["last_result"] = res

    bo = np.asarray(bo, np.float32)
    out = np.empty((B, S, E), np.float32)
    for b in range(B):
        out[b] = res.results[2 * b]["out"] + res.results[2 * b + 1]["out"] + bo
    return out


# revision 14
# speedup vs baseline: 1.0063x; 1.0063x over previous
"""Causal self-attention (B=4, S=2048, E=1024, H=16) on 8 trn2 NeuronCores.

Sharding: data parallel over batch (4) x tensor parallel over head groups (2).
Core c handles batch c//2, heads [ (c%2)*8, (c%2)*8+8 ).  Each core computes
its group's QKV projections, causal attention, and a partial output
projection; the host sums the two group partials per batch and adds bo.

Datapath: fp32r (relaxed fp32, 1 moving column/cycle) everywhere except
QT/KT, which are stored fp16.  The PE's moving-operand SBUF port is ~4
bytes/partition/cycle shared across concurrent matmuls, so the head-pair
score matmuls (row groups 0-63 / 64-127, issued back-to-back) stream both
fp16 operands at full rate instead of halving each other.  Scales fold as
Q/64, K/8, so PSUM scores arrive pre-divided by 64 for the exp paths.

exp is split across two engines to un-bottleneck the attention phase:
  - ScalarE: activation(Exp, scale=64) — all chain-critical (single-qb)
    tiles plus most others.
  - VectorE: custom-DVE pair EXPPOLY (deg-3 Taylor of e^y) then SQ6 (six
    squarings), exp(64 s) = poly(s)^64 at ~1e-4 rel err, on a rotating
    subset of double-qb tiles where extra latency hides.
Diagonal k-tiles skip the fully-masked left columns everywhere: exp, the
GpSimd triangle multiply, and the AV matmul all operate on the live column
range only (no zeroing pass).

The attention loop is q-block-pair-major (qp outer, head-pair inner), so
once qp=0 finishes every pair, the first half of the output projection is
interleaved into the qp=1 instruction stream: its PSUM tiles share the
attention accumulator tag and its PSUM->SBUF drains run on GpSimd, filling
PE gaps left by the score->exp->score dependency chain.

The softmax denominator comes from a 65th ones-column in the AV stationary
operand; the raw PSUM row is DMA-bounced through DRAM to broadcast it to 64
partitions, then inverted with the single-op custom-DVE fast reciprocal.
"""

import numpy as np

import concourse.mybir as mybir
import concourse.tile as tile
from concourse import bacc
from concourse.bass_utils import run_bass_kernel_spmd

F32 = mybir.dt.float32
F32R = mybir.dt.float32r
F16 = mybir.dt.float16
Exp = mybir.ActivationFunctionType.Exp
MULT = mybir.AluOpType.mult
ADD = mybir.AluOpType.add

B, S, E, H = 4, 2048, 1024, 16
D = 64          # head dim
HG = 8          # heads per core
G = 512         # group feature width
P = 128
NKT = S // P    # 16 k-tiles
NST = S // P    # 16 s-tiles
QB = 512        # q-block width
NQB = S // QB   # 4
ESUB = E // P   # 8
VW = D + 1      # V stationary width (64 dims + ones column)

_CACHE = {}


def _register_dve_ops():
    """exp(64*y) = (1 + y + y^2/2 + y^3/6)^64 as two custom-DVE ops."""
    if "dve" in _CACHE:
        return _CACHE["dve"]
    from concourse.dve_spec import Spec, Src0, C0, C1, One, sq, lower
    import concourse.dve_ops as dops

    def mk(name, spec):
        for op in dops.OPS:
            if op.name == name:
                return op
        sha = {}
        for ver in ("v3", "v4"):
            s = dops.DveOpSpec(name=name, opcode=0, uops=lower(spec, ver=ver),
                               rd1_en=False)
            sha[ver] = s.sha(ver)
        op = dops.DveOp(name, spec, subdim=False, uops_sha=sha)
        dops.OPS.append(op)
        dops.CUSTOM_DVE_SPECS[name] = spec
        dops._SUB_OPCODE_FOR_NAME[name] = (
            dops._CUSTOM_DVE_ROW_BASE + len(dops.OPS) - 1
        )
        return op

    exppoly = mk("EXPPOLY_ANT", Spec(
        body=((Src0 * C0 + C1) * Src0 + One) * Src0 + One,
        reference=lambda in0, s0, s1: ((in0 * s0 + s1) * in0 + 1) * in0 + 1,
    ))
    sq6 = mk("SQ6_ANT", Spec(
        body=sq(sq(sq(sq(sq(sq(Src0)))))),
        reference=lambda in0: in0 ** 64,
    ))
    _CACHE["dve"] = (exppoly, sq6)
    return exppoly, sq6


def _build_program():
    EXPPOLY, SQ6 = _register_dve_ops()
    nc = bacc.Bacc("TRN2", target_bir_lowering=False, debug=False)

    xt_d = nc.dram_tensor("xt", [E, S], F32R, kind="ExternalInput").ap()
    wq_d = nc.dram_tensor("wq", [E, G], F32R, kind="ExternalInput").ap()
    wk_d = nc.dram_tensor("wk", [E, G], F32R, kind="ExternalInput").ap()
    wv_d = nc.dram_tensor("wv", [E, G], F32R, kind="ExternalInput").ap()
    wo_d = nc.dram_tensor("wo", [G, E], F32R, kind="ExternalInput").ap()
    bq_d = nc.dram_tensor("bq", [P, 4], F32, kind="ExternalInput").ap()
    bk_d = nc.dram_tensor("bk", [P, 4], F32, kind="ExternalInput").ap()
    bv_d = nc.dram_tensor("bv", [P, G], F32, kind="ExternalInput").ap()
    tri_d = nc.dram_tensor("tri", [P, P], F32R, kind="ExternalInput").ap()
    one_d = nc.dram_tensor("one", [P, D], F32R, kind="ExternalInput").ap()
    out_d = nc.dram_tensor("out", [S, E], F32, kind="ExternalOutput").ap()
    # scratch for the reciprocal-row broadcast bounce
    rc_d = nc.dram_tensor("rc_scratch", [HG, NQB, QB], F32, kind="Internal").ap()

    qt_sb = nc.alloc_sbuf_tensor("qt_sb", [P, 4, S], F16).ap()
    kt_sb = nc.alloc_sbuf_tensor("kt_sb", [P, 4, S], F16).ap()
    vx_sb = nc.alloc_sbuf_tensor("vx_sb", [P, NKT, HG, VW], F32R).ap()
    tri_sb = nc.alloc_sbuf_tensor("tri_sb", [P, P], F32R).ap()
    ones_sb = nc.alloc_sbuf_tensor("ones_sb", [P, D], F32R).ap()
    bq_sb = nc.alloc_sbuf_tensor("bq_sb", [P, 4], F32).ap()
    bk_sb = nc.alloc_sbuf_tensor("bk_sb", [P, 4], F32).ap()
    bv_sb = nc.alloc_sbuf_tensor("bv_sb", [P, G], F32).ap()

    with tile.TileContext(nc) as tc:
        xt_r = xt_d.rearrange("(o p) s -> p o s", p=P)

        # ---- QKV projections (xT streamed per 512-token chunk) ----
        with (
            tc.tile_pool(name="w_pool", bufs=1) as wp,
            tc.tile_pool(name="xt_pool", bufs=2) as xp,
            tc.tile_pool(name="proj_ps", bufs=2, space="PSUM") as pp,
        ):
            wqp = wp.tile([P, ESUB, G], F32R, tag="wq", name="wqp")
            wkp = wp.tile([P, ESUB, G], F32R, tag="wk", name="wkp")
            wvp = wp.tile([P, ESUB, G], F32R, tag="wv", name="wvp")
            # first xT chunk loads ahead of the weights so the PE starts
            # sooner; wq fully precedes wk/wv so Q matmuls start first
            xtp0 = xp.tile([P, ESUB, QB], F32R, tag="xt", name="xtp")
            for e in range(ESUB):
                nc.sync.dma_start(
                    wqp[:, e, :], wq_d.rearrange("(o p) f -> p o f", p=P)[:, e, :]
                )
                nc.sync.dma_start(xtp0[:, e, :], xt_r[:, e, 0:QB])
            nc.sync.dma_start(bq_sb[:], bq_d[:])
            for e in range(ESUB):
                nc.sync.dma_start(
                    wkp[:, e, :], wk_d.rearrange("(o p) f -> p o f", p=P)[:, e, :]
                )
            nc.sync.dma_start(bk_sb[:], bk_d[:])
            for e in range(ESUB):
                nc.sync.dma_start(
                    wvp[:, e, :], wv_d.rearrange("(o p) f -> p o f", p=P)[:, e, :]
                )
            nc.sync.dma_start(bv_sb[:], bv_d[:])
            nc.sync.dma_start(tri_sb[:], tri_d[:])
            nc.sync.dma_start(ones_sb[:], one_d[:])
            for qb in range(NQB):
                if qb == 0:
                    xtp = xtp0
                else:
                    xtp = xp.tile([P, ESUB, QB], F32R, tag="xt", name="xtp")
                    nc.sync.dma_start(xtp[:], xt_r[:, :, qb * QB : (qb + 1) * QB])
                for cc in range(4):
                    q_ps = pp.tile([P, QB], F32, tag="q", name="q_ps")
                    for e in range(ESUB):
                        nc.tensor.matmul(
                            q_ps[:],
                            lhsT=wqp[:, e, cc * P : (cc + 1) * P],
                            rhs=xtp[:, e, :],
                            start=(e == 0),
                            stop=(e == ESUB - 1),
                        )
                    # fold bias and the 1/64 share of the softmax scale into Q
                    # (stored fp16; K carries the other 1/8)
                    nc.vector.tensor_scalar(
                        qt_sb[:, cc, qb * QB : (qb + 1) * QB],
                        q_ps[:],
                        bq_sb[:, cc : cc + 1],
                        1.0 / 64.0,
                        ADD,
                        MULT,
                    )
                    k_ps = pp.tile([P, QB], F32, tag="k", name="k_ps")
                    for e in range(ESUB):
                        nc.tensor.matmul(
                            k_ps[:],
                            lhsT=wkp[:, e, cc * P : (cc + 1) * P],
                            rhs=xtp[:, e, :],
                            start=(e == 0),
                            stop=(e == ESUB - 1),
                        )
                    nc.vector.tensor_scalar(
                        kt_sb[:, cc, qb * QB : (qb + 1) * QB],
                        k_ps[:],
                        bk_sb[:, cc : cc + 1],
                        1.0 / 8.0,
                        ADD,
                        MULT,
                    )
                for stl in range(4):
                    st = qb * 4 + stl
                    v_ps = pp.tile([P, G], F32, tag="v", name="v_ps")
                    for e in range(ESUB):
                        nc.tensor.matmul(
                            v_ps[:],
                            lhsT=xtp[:, e, stl * P : (stl + 1) * P],
                            rhs=wvp[:, e, :],
                            start=(e == 0),
                            stop=(e == ESUB - 1),
                        )
                    nc.vector.tensor_tensor(
                        vx_sb[:, st, :, 0:D],
                        v_ps.rearrange("p (h d) -> p h d", d=D),
                        bv_sb.rearrange("p (h d) -> p h d", d=D),
                        ADD,
                    )
                    nc.vector.tensor_copy(
                        vx_sb[:, st, :, D : D + 1],
                        ones_sb[:, 0:HG].rearrange("p (h u) -> p h u", u=1),
                    )

        # ---- attention + output projection ----
        with tc.tile_pool(name="at_pool", bufs=1) as atp:
            at_t = atp.tile([P, 4, S], F32R, name="at_t")
            # wo lives in the same long-lived pool and loads during attention
            wop = atp.tile([P, 4, E], F32R, name="wop")
            nc.sync.dma_start(wop[:], wo_d.rearrange("(o p) n -> p o n", p=P))
            with (
                tc.tile_pool(name="attn_ps", bufs=1, space="PSUM") as ap,
                tc.tile_pool(name="attn_sb", bufs=2) as sp,
            ):
                # heads run in even/odd pairs: the pair's score matmuls use
                # partition bases 0/64 (distinct PE row groups -> concurrent).
                # AV matmuls lag the scores by AV_LAG k-tiles so the PE never
                # waits on exp/mask; accumulators are per-(head, q-block) so
                # normalization runs mid-pass and frees PSUM slots early.
                AV_LAG = 4
                dbl_ctr = [0]

                def emit_outproj(st, n, drain):
                    o_ps = ap.tile([P, QB], F32, tag="a", name="o_ps", bufs=4)
                    for t in range(4):
                        nc.tensor.matmul(
                            o_ps[:],
                            lhsT=at_t[:, t, st * P : (st + 1) * P],
                            rhs=wop[:, t, n * QB : (n + 1) * QB],
                            start=(t == 0),
                            stop=(t == 3),
                        )
                    o_sb = sp.tile([P, QB], F32, tag="ost", name="o_sb", bufs=3)
                    if drain == "vector":
                        nc.vector.tensor_copy(o_sb[:], o_ps[:])
                    else:
                        nc.scalar.copy(o_sb[:], o_ps[:])
                    nc.sync.dma_start(
                        out_d[st * P : (st + 1) * P, n * QB : (n + 1) * QB],
                        o_sb[:],
                    )

                # output projection tiles st<8 interleave into the qb=2/3
                # rounds, st 8-11 into qb=3 (inputs ready block-granularity;
                # emitted only once their at_t inputs are certainly written,
                # so they never head-of-line-block the in-order PE queue)
                feed_a = [(st, n) for st in range(8) for n in range(2)]
                feed_b = [(st, n) for st in range(8, 12) for n in range(2)]

                norm_done = [0, 0, 0, 0]

                def normalize(ctx, h, qb):
                    a_t, heads, sub = ctx
                    norm_done[qb] += 1
                    hb = (h % 2) * D
                    a_ps = a_t[h]
                    # broadcast the RAW denominator row to 64 partitions via a
                    # DRAM bounce (a DRAM source AP may repeat along
                    # partitions, SBUF cannot), then take the reciprocal at
                    # partition base 0 with the fast custom-DVE op (~3e-6 rel
                    # err; it mishandles nonzero partition bases, hence this
                    # order)
                    dn = sp.tile([VW, QB], F32, tag="lg", name="dn", bufs=2)
                    nc.vector.tensor_copy(dn[D:VW, :], a_ps[D:VW, :])
                    nc.sync.dma_start(rc_d[h, qb : qb + 1, :], dn[D:VW, :])
                    db = sp.tile([D, QB], F32, tag="rs", name="db", bufs=2)
                    nc.sync.dma_start(
                        db[:], rc_d[h, qb : qb + 1, :].to_broadcast([D, QB])
                    )
                    rb_sb = sp.tile([D, QB], F32, tag="rbs", name="rb_sb", bufs=2)
                    nc.vector.reciprocal_approx_fast(rb_sb[:], db[:])
                    at_slice = at_t[hb : hb + D, sub, qb * QB : (qb + 1) * QB]
                    if hb == 0:
                        nc.vector.tensor_tensor(at_slice, a_ps[0:D, :], rb_sb[:], MULT)
                    else:
                        tmp = sp.tile([D, QB], F32R, tag="tmp", name="tmp", bufs=2)
                        nc.vector.tensor_tensor(tmp[:], a_ps[0:D, :], rb_sb[:], MULT)
                        nc.sync.dma_start(at_slice, tmp[:])

                def av_main(ctx, pt, kt, qb):
                    # AV over the columns with no triangle-mask dependency:
                    # everything right of the diagonal 128-block (diag k-tile)
                    # or the whole live range (plain k-tile)
                    a_t, heads, sub = ctx
                    if kt // 4 == qb:
                        off = P * (kt % 4 + 1)
                    else:
                        off = 0
                    if off >= QB:
                        return
                    for h in heads:
                        nc.tensor.matmul(
                            a_t[h][0:VW, off:],
                            lhsT=vx_sb[:, kt, h, :],
                            rhs=pt[:, h % 2, off:],
                            start=(kt == 0),
                            stop=False,
                        )

                def av_tri(ctx, pt, kt, qb):
                    # the diagonal 128-column block, gated on the GpSimd
                    # triangle multiply; lags further so the in-order PE queue
                    # never stalls on it.  Carries start (kt==0, runs first
                    # into a fresh bank only for qb==0) and stop/normalize on
                    # the final k-tile.
                    a_t, heads, sub = ctx
                    m = kt % 4
                    last = kt == 4 * qb + 3
                    for h in heads:
                        nc.tensor.matmul(
                            a_t[h][0:VW, P * m : P * (m + 1)],
                            lhsT=vx_sb[:, kt, h, :],
                            rhs=pt[:, h % 2, P * m : P * (m + 1)],
                            start=False,
                            stop=last,
                        )
                    if last:
                        for h in heads:
                            normalize(ctx, h, qb)

                # one software pipeline across all (qb, pair) blocks: the AV/
                # normalize drain of each block interleaves with the next
                # block's score matmuls instead of stalling the in-order PE
                AV_LAG_TRI = 7
                pend_main = []   # (ctx, pt, kt, qb)
                pend_tri = []    # (ctx, pt, kt, qb)
                step = [0]

                def pump():
                    if len(pend_main) > AV_LAG:
                        av_main(*pend_main.pop(0))
                    if len(pend_tri) > AV_LAG_TRI:
                        av_tri(*pend_tri.pop(0))

                for qb in range(NQB):
                    rstep = 0
                    for pair in range(4):
                        heads = (2 * pair, 2 * pair + 1)
                        sub = pair
                        a_t = {
                            h: ap.tile([P, QB], F32, tag="a", name="a_ps", bufs=4)
                            for h in heads
                        }
                        ctx = (a_t, heads, sub)
                        for kt in range(4 * qb + 4):
                            s_t = ap.tile(
                                [P, 2, QB], F32, tag="s", name="s_ps", bufs=2
                            )
                            for h in heads:
                                hb = (h % 2) * D
                                nc.tensor.matmul(
                                    s_t[:, h % 2, :],
                                    lhsT=kt_sb[
                                        hb : hb + D, sub, kt * P : (kt + 1) * P
                                    ],
                                    rhs=qt_sb[
                                        hb : hb + D, sub, qb * QB : (qb + 1) * QB
                                    ],
                                    start=True,
                                    stop=True,
                                )
                            pt = sp.tile(
                                [P, 2, QB], F32R, tag="pt", name="pt", bufs=10
                            )
                            diag = kt // 4 == qb
                            m = kt % 4 if diag else 0
                            # exp engine split: rotate a fixed share of tiles
                            # onto VectorE (two-op poly^64), rest on ScalarE
                            step[0] += 1
                            if step[0] % 5 == 2 and not diag:
                                tx = sp.tile(
                                    [P, 2, QB], F32, tag="tx", name="tx", bufs=2
                                )
                                nc.vector._custom_dve(
                                    EXPPOLY,
                                    out=tx[:, :, P * m :],
                                    in0=s_t[:, :, P * m :],
                                    s0=1.0 / 6.0,
                                    s1=0.5,
                                )
                                nc.vector._custom_dve(
                                    SQ6,
                                    out=pt[:, :, P * m :],
                                    in0=tx[:, :, P * m :],
                                )
                            else:
                                nc.scalar.activation(
                                    pt[:, :, P * m :], s_t[:, :, P * m :],
                                    Exp, scale=64.0,
                                )
                            if diag:
                                for h in heads:
                                    nc.gpsimd.tensor_tensor(
                                        pt[:, h % 2, P * m : P * (m + 1)],
                                        pt[:, h % 2, P * m : P * (m + 1)],
                                        tri_sb[:],
                                        MULT,
                                    )
                                pend_tri.append((ctx, pt, kt, qb))
                            pend_main.append((ctx, pt, kt, qb))
                            pump()
                            rstep += 1
                            if (
                                qb >= 2
                                and feed_a
                                and rstep % 3 == 0
                                and norm_done[feed_a[0][0] // 4] == 8
                            ):
                                emit_outproj(*feed_a.pop(0), drain="scalar")
                            elif (
                                qb == 3
                                and feed_b
                                and rstep % 4 == 2
                                and norm_done[feed_b[0][0] // 4] == 8
                            ):
                                emit_outproj(*feed_b.pop(0), drain="scalar")
                while pend_main or pend_tri:
                    if pend_main:
                        av_main(*pend_main.pop(0))
                    if pend_tri and (
                        not pend_main
                        or len(pend_tri) > AV_LAG_TRI - AV_LAG
                    ):
                        av_tri(*pend_tri.pop(0))
                for st_n in feed_a + feed_b:
                    emit_outproj(*st_n, drain="scalar")

            # ---- second-half output projection (first half ran inside the
            # attention loop; host adds the other group's partial + bo) ----
            with (
                tc.tile_pool(name="op_ps", bufs=2, space="PSUM") as op,
                tc.tile_pool(name="op_sb", bufs=3) as osp,
            ):
                for st in range(12, NST):
                    for n in range(2):
                        o_ps = op.tile([P, QB], F32, tag="o", name="o_ps")
                        for t in range(4):
                            nc.tensor.matmul(
                                o_ps[:],
                                lhsT=at_t[:, t, st * P : (st + 1) * P],
                                rhs=wop[:, t, n * QB : (n + 1) * QB],
                                start=(t == 0),
                                stop=(t == 3),
                            )
                        o_sb = osp.tile([P, QB], F32, tag="ost", name="o_sb")
                        nc.scalar.copy(o_sb[:], o_ps[:])
                        nc.sync.dma_start(
                            out_d[st * P : (st + 1) * P, n * QB : (n + 1) * QB],
                            o_sb[:],
                        )

    nc.compile()
    return nc


def _prep_inputs(x, Wqkv, bqkv, Wo, bo):
    x = np.asarray(x, np.float32)
    Wqkv = np.asarray(Wqkv, np.float32)
    bqkv = np.asarray(bqkv, np.float32)
    Wo = np.asarray(Wo, np.float32)

    # 128x128 inclusive lower-triangle-in-(q,k) == kl <= ql in [k, q] layout
    kl = np.arange(P)[:, None]
    tri = (kl <= np.arange(P)[None, :]).astype(np.float32)


    in_maps = []
    for c in range(8):
        b, g = divmod(c, 2)
        lo, hi = g * G, (g + 1) * G
        in_maps.append(
            {
                "xt": np.ascontiguousarray(x[b].T),
                "wq": np.ascontiguousarray(Wqkv[:, lo:hi]),
                "wk": np.ascontiguousarray(Wqkv[:, E + lo : E + hi]),
                "wv": np.ascontiguousarray(Wqkv[:, 2 * E + lo : 2 * E + hi]),
                "wo": np.ascontiguousarray(Wo[lo:hi, :]),
                "bq": np.ascontiguousarray(bqkv[lo:hi].reshape(4, P).T),
                "bk": np.ascontiguousarray(bqkv[E + lo : E + hi].reshape(4, P).T),
                "bv": np.tile(bqkv[2 * E + lo : 2 * E + hi][None, :], (P, 1)).astype(
                    np.float32
                ),
                "tri": tri,
                "one": np.ones((P, D), np.float32),
            }
        )
    return in_maps


def kernel(x, Wqkv, bqkv, Wo, bo, _trace=False):
    if "nc" not in _CACHE:
        _CACHE["nc"] = _build_program()
    nc = _CACHE["nc"]

    in_maps = _prep_inputs(x, Wqkv, bqkv, Wo, bo)
    res = run_bass_kernel_spmd(nc, in_maps, core_ids=list(range(8)), trace=_trace)
    _CACHE["last_result"] = res

    bo = np.asarray(bo, np.float32)
    out = np.empty((B, S, E), np.float32)
    for b in range(B):
        out[b] = res.results[2 * b]["out"] + res.results[2 * b + 1]["out"] + bo
    return out


# revision 15
# speedup vs baseline: 1.0225x; 1.0161x over previous
"""Causal self-attention (B=4, S=2048, E=1024, H=16) on 8 trn2 NeuronCores.

Sharding: data parallel over batch (4) x tensor parallel over head groups (2).
Core c handles batch c//2, heads [ (c%2)*8, (c%2)*8+8 ).  Each core computes
its group's QKV projections, causal attention, and a partial output
projection; the host sums the two group partials per batch and adds bo.

Datapath: fp32r (relaxed fp32, 1 moving column/cycle) everywhere except
QT/KT, which are stored fp16.  The PE's moving-operand SBUF port is ~4
bytes/partition/cycle shared across concurrent matmuls, so the head-pair
score matmuls (row groups 0-63 / 64-127, issued back-to-back) stream both
fp16 operands at full rate instead of halving each other.  Scales fold as
Q/64, K/8, so PSUM scores arrive pre-divided by 64 for the exp paths.

exp is split across two engines to un-bottleneck the attention phase:
  - ScalarE: activation(Exp, scale=64) — all chain-critical (single-qb)
    tiles plus most others.
  - VectorE: custom-DVE pair EXPPOLY (deg-3 Taylor of e^y) then SQ6 (six
    squarings), exp(64 s) = poly(s)^64 at ~1e-4 rel err, on a rotating
    subset of double-qb tiles where extra latency hides.
Diagonal k-tiles skip the fully-masked left columns everywhere: exp, the
GpSimd triangle multiply, and the AV matmul all operate on the live column
range only (no zeroing pass).

The attention loop is q-block-pair-major (qp outer, head-pair inner), so
once qp=0 finishes every pair, the first half of the output projection is
interleaved into the qp=1 instruction stream: its PSUM tiles share the
attention accumulator tag and its PSUM->SBUF drains run on GpSimd, filling
PE gaps left by the score->exp->score dependency chain.

The softmax denominator comes from a 65th ones-column in the AV stationary
operand; the raw PSUM row is DMA-bounced through DRAM to broadcast it to 64
partitions, then inverted with the single-op custom-DVE fast reciprocal.
"""

import numpy as np

import concourse.mybir as mybir
import concourse.tile as tile
from concourse import bacc
from concourse.bass_utils import run_bass_kernel_spmd

F32 = mybir.dt.float32
F32R = mybir.dt.float32r
F16 = mybir.dt.float16
Exp = mybir.ActivationFunctionType.Exp
MULT = mybir.AluOpType.mult
ADD = mybir.AluOpType.add

B, S, E, H = 4, 2048, 1024, 16
D = 64          # head dim
HG = 8          # heads per core
G = 512         # group feature width
P = 128
NKT = S // P    # 16 k-tiles
NST = S // P    # 16 s-tiles
QB = 512        # q-block width
NQB = S // QB   # 4
ESUB = E // P   # 8
VW = D + 1      # V stationary width (64 dims + ones column)

_CACHE = {}


def _register_dve_ops():
    """exp(64*y) = (1 + y + y^2/2 + y^3/6)^64 as two custom-DVE ops."""
    if "dve" in _CACHE:
        return _CACHE["dve"]
    from concourse.dve_spec import Spec, Src0, C0, C1, One, sq, lower
    import concourse.dve_ops as dops

    def mk(name, spec):
        for op in dops.OPS:
            if op.name == name:
                return op
        sha = {}
        for ver in ("v3", "v4"):
            s = dops.DveOpSpec(name=name, opcode=0, uops=lower(spec, ver=ver),
                               rd1_en=False)
            sha[ver] = s.sha(ver)
        op = dops.DveOp(name, spec, subdim=False, uops_sha=sha)
        dops.OPS.append(op)
        dops.CUSTOM_DVE_SPECS[name] = spec
        dops._SUB_OPCODE_FOR_NAME[name] = (
            dops._CUSTOM_DVE_ROW_BASE + len(dops.OPS) - 1
        )
        return op

    exppoly = mk("EXPPOLY_ANT", Spec(
        body=((Src0 * C0 + C1) * Src0 + One) * Src0 + One,
        reference=lambda in0, s0, s1: ((in0 * s0 + s1) * in0 + 1) * in0 + 1,
    ))
    sq6 = mk("SQ6_ANT", Spec(
        body=sq(sq(sq(sq(sq(sq(Src0)))))),
        reference=lambda in0: in0 ** 64,
    ))
    _CACHE["dve"] = (exppoly, sq6)
    return exppoly, sq6


def _build_program():
    EXPPOLY, SQ6 = _register_dve_ops()
    nc = bacc.Bacc("TRN2", target_bir_lowering=False, debug=False)

    xt_d = nc.dram_tensor("xt", [E, S], F32R, kind="ExternalInput").ap()
    wq_d = nc.dram_tensor("wq", [E, G], F32R, kind="ExternalInput").ap()
    wk_d = nc.dram_tensor("wk", [E, G], F32R, kind="ExternalInput").ap()
    wv_d = nc.dram_tensor("wv", [E, G], F32R, kind="ExternalInput").ap()
    wo_d = nc.dram_tensor("wo", [G, E], F32R, kind="ExternalInput").ap()
    bq_d = nc.dram_tensor("bq", [P, 4], F32, kind="ExternalInput").ap()
    bk_d = nc.dram_tensor("bk", [P, 4], F32, kind="ExternalInput").ap()
    bv_d = nc.dram_tensor("bv", [P, G], F32, kind="ExternalInput").ap()
    tri_d = nc.dram_tensor("tri", [P, P], F32R, kind="ExternalInput").ap()
    one_d = nc.dram_tensor("one", [P, D], F32R, kind="ExternalInput").ap()
    out_d = nc.dram_tensor("out", [S, E], F32, kind="ExternalOutput").ap()
    # scratch for the reciprocal-row broadcast bounce
    rc_d = nc.dram_tensor("rc_scratch", [HG, NQB, QB], F32, kind="Internal").ap()

    qt_sb = nc.alloc_sbuf_tensor("qt_sb", [P, 4, S], F16).ap()
    kt_sb = nc.alloc_sbuf_tensor("kt_sb", [P, 4, S], F16).ap()
    vx_sb = nc.alloc_sbuf_tensor("vx_sb", [P, NKT, HG, VW], F32R).ap()
    tri_sb = nc.alloc_sbuf_tensor("tri_sb", [P, P], F32R).ap()
    ones_sb = nc.alloc_sbuf_tensor("ones_sb", [P, D], F32R).ap()
    bq_sb = nc.alloc_sbuf_tensor("bq_sb", [P, 4], F32).ap()
    bk_sb = nc.alloc_sbuf_tensor("bk_sb", [P, 4], F32).ap()
    bv_sb = nc.alloc_sbuf_tensor("bv_sb", [P, G], F32).ap()

    with tile.TileContext(nc) as tc:
        xt_r = xt_d.rearrange("(o p) s -> p o s", p=P)

        # ---- QKV projections (xT streamed per 512-token chunk) ----
        with (
            tc.tile_pool(name="w_pool", bufs=1) as wp,
            tc.tile_pool(name="xt_pool", bufs=2) as xp,
            tc.tile_pool(name="proj_ps", bufs=2, space="PSUM") as pp,
        ):
            wqp = wp.tile([P, ESUB, G], F32R, tag="wq", name="wqp")
            wkp = wp.tile([P, ESUB, G], F32R, tag="wk", name="wkp")
            wvp = wp.tile([P, ESUB, G], F32R, tag="wv", name="wvp")
            # first xT chunk loads ahead of the weights so the PE starts
            # sooner; wq fully precedes wk/wv so Q matmuls start first
            xtp0 = xp.tile([P, ESUB, QB], F32R, tag="xt", name="xtp")
            nc.sync.dma_start(xtp0[:], xt_r[:, :, 0:QB])
            for e in range(ESUB):
                nc.sync.dma_start(
                    wqp[:, e, :], wq_d.rearrange("(o p) f -> p o f", p=P)[:, e, :]
                )
            nc.sync.dma_start(bq_sb[:], bq_d[:])
            for e in range(ESUB):
                nc.sync.dma_start(
                    wkp[:, e, :], wk_d.rearrange("(o p) f -> p o f", p=P)[:, e, :]
                )
            nc.sync.dma_start(bk_sb[:], bk_d[:])
            for e in range(ESUB):
                nc.sync.dma_start(
                    wvp[:, e, :], wv_d.rearrange("(o p) f -> p o f", p=P)[:, e, :]
                )
            nc.sync.dma_start(bv_sb[:], bv_d[:])
            nc.sync.dma_start(tri_sb[:], tri_d[:])
            nc.sync.dma_start(ones_sb[:], one_d[:])
            for qb in range(NQB):
                if qb == 0:
                    xtp = xtp0
                else:
                    xtp = xp.tile([P, ESUB, QB], F32R, tag="xt", name="xtp")
                    nc.sync.dma_start(xtp[:], xt_r[:, :, qb * QB : (qb + 1) * QB])
                for cc in range(4):
                    q_ps = pp.tile([P, QB], F32, tag="q", name="q_ps")
                    for e in range(ESUB):
                        nc.tensor.matmul(
                            q_ps[:],
                            lhsT=wqp[:, e, cc * P : (cc + 1) * P],
                            rhs=xtp[:, e, :],
                            start=(e == 0),
                            stop=(e == ESUB - 1),
                        )
                    # fold bias and the 1/64 share of the softmax scale into Q
                    # (stored fp16; K carries the other 1/8)
                    nc.vector.tensor_scalar(
                        qt_sb[:, cc, qb * QB : (qb + 1) * QB],
                        q_ps[:],
                        bq_sb[:, cc : cc + 1],
                        1.0 / 64.0,
                        ADD,
                        MULT,
                    )
                    k_ps = pp.tile([P, QB], F32, tag="k", name="k_ps")
                    for e in range(ESUB):
                        nc.tensor.matmul(
                            k_ps[:],
                            lhsT=wkp[:, e, cc * P : (cc + 1) * P],
                            rhs=xtp[:, e, :],
                            start=(e == 0),
                            stop=(e == ESUB - 1),
                        )
                    nc.vector.tensor_scalar(
                        kt_sb[:, cc, qb * QB : (qb + 1) * QB],
                        k_ps[:],
                        bk_sb[:, cc : cc + 1],
                        1.0 / 8.0,
                        ADD,
                        MULT,
                    )
                for stl in range(4):
                    st = qb * 4 + stl
                    v_ps = pp.tile([P, G], F32, tag="v", name="v_ps")
                    for e in range(ESUB):
                        nc.tensor.matmul(
                            v_ps[:],
                            lhsT=xtp[:, e, stl * P : (stl + 1) * P],
                            rhs=wvp[:, e, :],
                            start=(e == 0),
                            stop=(e == ESUB - 1),
                        )
                    nc.vector.tensor_tensor(
                        vx_sb[:, st, :, 0:D],
                        v_ps.rearrange("p (h d) -> p h d", d=D),
                        bv_sb.rearrange("p (h d) -> p h d", d=D),
                        ADD,
                    )
                    nc.vector.tensor_copy(
                        vx_sb[:, st, :, D : D + 1],
                        ones_sb[:, 0:HG].rearrange("p (h u) -> p h u", u=1),
                    )

        # ---- attention + output projection ----
        with tc.tile_pool(name="at_pool", bufs=1) as atp:
            at_t = atp.tile([P, 4, S], F32R, name="at_t")
            # wo lives in the same long-lived pool and loads during attention
            wop = atp.tile([P, 4, E], F32R, name="wop")
            nc.sync.dma_start(wop[:], wo_d.rearrange("(o p) n -> p o n", p=P))
            with (
                tc.tile_pool(name="attn_ps", bufs=1, space="PSUM") as ap,
                tc.tile_pool(name="attn_sb", bufs=2) as sp,
            ):
                # heads run in even/odd pairs: the pair's score matmuls use
                # partition bases 0/64 (distinct PE row groups -> concurrent).
                # AV matmuls lag the scores by AV_LAG k-tiles so the PE never
                # waits on exp/mask; accumulators are per-(head, q-block) so
                # normalization runs mid-pass and frees PSUM slots early.
                AV_LAG = 4
                dbl_ctr = [0]

                def emit_outproj(st, n, drain):
                    o_ps = ap.tile([P, QB], F32, tag="a", name="o_ps", bufs=4)
                    for t in range(4):
                        nc.tensor.matmul(
                            o_ps[:],
                            lhsT=at_t[:, t, st * P : (st + 1) * P],
                            rhs=wop[:, t, n * QB : (n + 1) * QB],
                            start=(t == 0),
                            stop=(t == 3),
                        )
                    o_sb = sp.tile([P, QB], F32, tag="ost", name="o_sb", bufs=3)
                    if drain == "vector":
                        nc.vector.tensor_copy(o_sb[:], o_ps[:])
                    else:
                        nc.scalar.copy(o_sb[:], o_ps[:])
                    nc.sync.dma_start(
                        out_d[st * P : (st + 1) * P, n * QB : (n + 1) * QB],
                        o_sb[:],
                    )

                # output projection tiles st<8 interleave into the qb=2/3
                # rounds, st 8-11 into qb=3 (inputs ready block-granularity;
                # emitted only once their at_t inputs are certainly written,
                # so they never head-of-line-block the in-order PE queue)
                feed_a = [(st, n) for st in range(8) for n in range(2)]
                feed_b = [(st, n) for st in range(8, 12) for n in range(2)]

                norm_done = [0, 0, 0, 0]

                def normalize(ctx, h, qb):
                    a_t, heads, sub = ctx
                    norm_done[qb] += 1
                    hb = (h % 2) * D
                    a_ps = a_t[h]
                    # broadcast the RAW denominator row to 64 partitions via a
                    # DRAM bounce (a DRAM source AP may repeat along
                    # partitions, SBUF cannot), then take the reciprocal at
                    # partition base 0 with the fast custom-DVE op (~3e-6 rel
                    # err; it mishandles nonzero partition bases, hence this
                    # order)
                    dn = sp.tile([VW, QB], F32, tag="lg", name="dn", bufs=2)
                    nc.vector.tensor_copy(dn[D:VW, :], a_ps[D:VW, :])
                    nc.sync.dma_start(rc_d[h, qb : qb + 1, :], dn[D:VW, :])
                    db = sp.tile([D, QB], F32, tag="rs", name="db", bufs=2)
                    nc.sync.dma_start(
                        db[:], rc_d[h, qb : qb + 1, :].to_broadcast([D, QB])
                    )
                    rb_sb = sp.tile([D, QB], F32, tag="rbs", name="rb_sb", bufs=2)
                    nc.vector.reciprocal_approx_fast(rb_sb[:], db[:])
                    at_slice = at_t[hb : hb + D, sub, qb * QB : (qb + 1) * QB]
                    if hb == 0:
                        nc.vector.tensor_tensor(at_slice, a_ps[0:D, :], rb_sb[:], MULT)
                    else:
                        tmp = sp.tile([D, QB], F32R, tag="tmp", name="tmp", bufs=2)
                        nc.vector.tensor_tensor(tmp[:], a_ps[0:D, :], rb_sb[:], MULT)
                        nc.sync.dma_start(at_slice, tmp[:])

                def av_main(ctx, pt, kt, qb):
                    # AV over the columns with no triangle-mask dependency:
                    # everything right of the diagonal 128-block (diag k-tile)
                    # or the whole live range (plain k-tile)
                    a_t, heads, sub = ctx
                    if kt // 4 == qb:
                        off = P * (kt % 4 + 1)
                    else:
                        off = 0
                    if off >= QB:
                        return
                    for h in heads:
                        nc.tensor.matmul(
                            a_t[h][0:VW, off:],
                            lhsT=vx_sb[:, kt, h, :],
                            rhs=pt[:, h % 2, off:],
                            start=(kt == 0),
                            stop=False,
                        )

                def av_tri(ctx, pt, kt, qb):
                    # the diagonal 128-column block, gated on the GpSimd
                    # triangle multiply; lags further so the in-order PE queue
                    # never stalls on it.  Carries start (kt==0, runs first
                    # into a fresh bank only for qb==0) and stop/normalize on
                    # the final k-tile.
                    a_t, heads, sub = ctx
                    m = kt % 4
                    last = kt == 4 * qb + 3
                    for h in heads:
                        nc.tensor.matmul(
                            a_t[h][0:VW, P * m : P * (m + 1)],
                            lhsT=vx_sb[:, kt, h, :],
                            rhs=pt[:, h % 2, P * m : P * (m + 1)],
                            start=False,
                            stop=last,
                        )
                    if last:
                        for h in heads:
                            normalize(ctx, h, qb)

                # one software pipeline across all (qb, pair) blocks: the AV/
                # normalize drain of each block interleaves with the next
                # block's score matmuls instead of stalling the in-order PE
                AV_LAG_TRI = 7
                pend_main = []   # (ctx, pt, kt, qb)
                pend_tri = []    # (ctx, pt, kt, qb)
                step = [0]

                def pump():
                    if len(pend_main) > AV_LAG:
                        av_main(*pend_main.pop(0))
                    if len(pend_tri) > AV_LAG_TRI:
                        av_tri(*pend_tri.pop(0))

                for qb in range(NQB):
                    rstep = 0
                    for pair in range(4):
                        heads = (2 * pair, 2 * pair + 1)
                        sub = pair
                        a_t = {
                            h: ap.tile([P, QB], F32, tag="a", name="a_ps", bufs=4)
                            for h in heads
                        }
                        ctx = (a_t, heads, sub)
                        for kt in range(4 * qb + 4):
                            s_t = ap.tile(
                                [P, 2, QB], F32, tag="s", name="s_ps", bufs=2
                            )
                            for h in heads:
                                hb = (h % 2) * D
                                nc.tensor.matmul(
                                    s_t[:, h % 2, :],
                                    lhsT=kt_sb[
                                        hb : hb + D, sub, kt * P : (kt + 1) * P
                                    ],
                                    rhs=qt_sb[
                                        hb : hb + D, sub, qb * QB : (qb + 1) * QB
                                    ],
                                    start=True,
                                    stop=True,
                                )
                            pt = sp.tile(
                                [P, 2, QB], F32R, tag="pt", name="pt", bufs=10
                            )
                            diag = kt // 4 == qb
                            m = kt % 4 if diag else 0
                            # exp engine split: rotate a fixed share of tiles
                            # onto VectorE (two-op poly^64), rest on ScalarE
                            step[0] += 1
                            if step[0] % 5 == 2 and not diag:
                                tx = sp.tile(
                                    [P, 2, QB], F32, tag="tx", name="tx", bufs=2
                                )
                                nc.vector._custom_dve(
                                    EXPPOLY,
                                    out=tx[:, :, P * m :],
                                    in0=s_t[:, :, P * m :],
                                    s0=1.0 / 6.0,
                                    s1=0.5,
                                )
                                nc.vector._custom_dve(
                                    SQ6,
                                    out=pt[:, :, P * m :],
                                    in0=tx[:, :, P * m :],
                                )
                            else:
                                nc.scalar.activation(
                                    pt[:, :, P * m :], s_t[:, :, P * m :],
                                    Exp, scale=64.0,
                                )
                            if diag:
                                for h in heads:
                                    nc.gpsimd.tensor_tensor(
                                        pt[:, h % 2, P * m : P * (m + 1)],
                                        pt[:, h % 2, P * m : P * (m + 1)],
                                        tri_sb[:],
                                        MULT,
                                    )
                                pend_tri.append((ctx, pt, kt, qb))
                            pend_main.append((ctx, pt, kt, qb))
                            pump()
                            rstep += 1
                            if (
                                qb >= 2
                                and feed_a
                                and rstep % 3 == 0
                                and norm_done[feed_a[0][0] // 4] == 8
                            ):
                                emit_outproj(*feed_a.pop(0), drain="scalar")
                            elif (
                                qb == 3
                                and feed_b
                                and rstep % 4 == 2
                                and norm_done[feed_b[0][0] // 4] == 8
                            ):
                                emit_outproj(*feed_b.pop(0), drain="scalar")
                while pend_main or pend_tri:
                    if pend_main:
                        av_main(*pend_main.pop(0))
                    if pend_tri and (
                        not pend_main
                        or len(pend_tri) > AV_LAG_TRI - AV_LAG
                    ):
                        av_tri(*pend_tri.pop(0))
                for st_n in feed_a + feed_b:
                    emit_outproj(*st_n, drain="scalar")

            # ---- second-half output projection (first half ran inside the
            # attention loop; host adds the other group's partial + bo) ----
            with (
                tc.tile_pool(name="op_ps", bufs=2, space="PSUM") as op,
                tc.tile_pool(name="op_sb", bufs=3) as osp,
            ):
                for st in range(12, NST):
                    for n in range(2):
                        o_ps = op.tile([P, QB], F32, tag="o", name="o_ps")
                        for t in range(4):
                            nc.tensor.matmul(
                                o_ps[:],
                                lhsT=at_t[:, t, st * P : (st + 1) * P],
                                rhs=wop[:, t, n * QB : (n + 1) * QB],
                                start=(t == 0),
                                stop=(t == 3),
                            )
                        o_sb = osp.tile([P, QB], F32, tag="ost", name="o_sb")
                        nc.scalar.copy(o_sb[:], o_ps[:])
                        nc.sync.dma_start(
                            out_d[st * P : (st + 1) * P, n * QB : (n + 1) * QB],
                            o_sb[:],
                        )

    nc.compile()
    return nc


def _prep_inputs(x, Wqkv, bqkv, Wo, bo):
    x = np.asarray(x, np.float32)
    Wqkv = np.asarray(Wqkv, np.float32)
    bqkv = np.asarray(bqkv, np.float32)
    Wo = np.asarray(Wo, np.float32)

    # 128x128 inclusive lower-triangle-in-(q,k) == kl <= ql in [k, q] layout
    kl = np.arange(P)[:, None]
    tri = (kl <= np.arange(P)[None, :]).astype(np.float32)


    in_maps = []
    for c in range(8):
        b, g = divmod(c, 2)
        lo, hi = g * G, (g + 1) * G
        in_maps.append(
            {
                "xt": np.ascontiguousarray(x[b].T),
                "wq": np.ascontiguousarray(Wqkv[:, lo:hi]),
                "wk": np.ascontiguousarray(Wqkv[:, E + lo : E + hi]),
                "wv": np.ascontiguousarray(Wqkv[:, 2 * E + lo : 2 * E + hi]),
                "wo": np.ascontiguousarray(Wo[lo:hi, :]),
                "bq": np.ascontiguousarray(bqkv[lo:hi].reshape(4, P).T),
                "bk": np.ascontiguousarray(bqkv[E + lo : E + hi].reshape(4, P).T),
                "bv": np.tile(bqkv[2 * E + lo : 2 * E + hi][None, :], (P, 1)).astype(
                    np.float32
                ),
                "tri": tri,
                "one": np.ones((P, D), np.float32),
            }
        )
    return in_maps


def kernel(x, Wqkv, bqkv, Wo, bo, _trace=False):
    if "nc" not in _CACHE:
        _CACHE["nc"] = _build_program()
    nc = _CACHE["nc"]

    in_maps = _prep_inputs(x, Wqkv, bqkv, Wo, bo)
    res = run_bass_kernel_spmd(nc, in_maps, core_ids=list(range(8)), trace=_trace)
    _CACHE["last_result"] = res

    bo = np.asarray(bo, np.float32)
    out = np.empty((B, S, E), np.float32)
    for b in range(B):
        out[b] = res.results[2 * b]["out"] + res.results[2 * b + 1]["out"] + bo
    return out


# revision 16
# speedup vs baseline: 1.0900x; 1.0660x over previous
"""Causal self-attention (B=4, S=2048, E=1024, H=16) on 8 trn2 NeuronCores.

Sharding: data parallel over batch (4) x tensor parallel over head groups (2).
Core c handles batch c//2, heads [ (c%2)*8, (c%2)*8+8 ).  Each core computes
its group's QKV projections, causal attention, and a partial output
projection; the host sums the two group partials per batch and adds bo.

Datapath: fp32r (relaxed fp32, 1 moving column/cycle) everywhere except
QT/KT, which are stored fp16.  The PE's moving-operand SBUF port is ~4
bytes/partition/cycle shared across concurrent matmuls, so the head-pair
score matmuls (row groups 0-63 / 64-127, issued back-to-back) stream both
fp16 operands at full rate instead of halving each other.  Scales fold as
Q/64, K/8, so PSUM scores arrive pre-divided by 64 for the exp paths.

exp is split across two engines to un-bottleneck the attention phase:
  - ScalarE: activation(Exp, scale=64) — all chain-critical (single-qb)
    tiles plus most others.
  - VectorE: custom-DVE pair EXPPOLY (deg-3 Taylor of e^y) then SQ6 (six
    squarings), exp(64 s) = poly(s)^64 at ~1e-4 rel err, on a rotating
    subset of double-qb tiles where extra latency hides.
Diagonal k-tiles skip the fully-masked left columns everywhere: exp, the
GpSimd triangle multiply, and the AV matmul all operate on the live column
range only (no zeroing pass).

The attention loop is q-block-pair-major (qp outer, head-pair inner), so
once qp=0 finishes every pair, the first half of the output projection is
interleaved into the qp=1 instruction stream: its PSUM tiles share the
attention accumulator tag and its PSUM->SBUF drains run on GpSimd, filling
PE gaps left by the score->exp->score dependency chain.

The softmax denominator comes from a 65th ones-column in the AV stationary
operand; the raw PSUM row is DMA-bounced through DRAM to broadcast it to 64
partitions, then inverted with the single-op custom-DVE fast reciprocal.
"""

import numpy as np

import concourse.mybir as mybir
import concourse.tile as tile
from concourse import bacc
from concourse.bass_utils import run_bass_kernel_spmd

F32 = mybir.dt.float32
F32R = mybir.dt.float32r
F16 = mybir.dt.float16
Exp = mybir.ActivationFunctionType.Exp
MULT = mybir.AluOpType.mult
ADD = mybir.AluOpType.add

B, S, E, H = 4, 2048, 1024, 16
D = 64          # head dim
HG = 8          # heads per core
G = 512         # group feature width
P = 128
NKT = S // P    # 16 k-tiles
NST = S // P    # 16 s-tiles
QB = 512        # q-block width
NQB = S // QB   # 4
ESUB = E // P   # 8
VW = D + 1      # V stationary width (64 dims + ones column)

_CACHE = {}


def _register_dve_ops():
    """exp(64*y) = (1 + y + y^2/2 + y^3/6)^64 as two custom-DVE ops."""
    if "dve" in _CACHE:
        return _CACHE["dve"]
    from concourse.dve_spec import Spec, Src0, C0, C1, One, sq, lower
    import concourse.dve_ops as dops

    def mk(name, spec):
        for op in dops.OPS:
            if op.name == name:
                return op
        sha = {}
        for ver in ("v3", "v4"):
            s = dops.DveOpSpec(name=name, opcode=0, uops=lower(spec, ver=ver),
                               rd1_en=False)
            sha[ver] = s.sha(ver)
        op = dops.DveOp(name, spec, subdim=False, uops_sha=sha)
        dops.OPS.append(op)
        dops.CUSTOM_DVE_SPECS[name] = spec
        dops._SUB_OPCODE_FOR_NAME[name] = (
            dops._CUSTOM_DVE_ROW_BASE + len(dops.OPS) - 1
        )
        return op

    exppoly = mk("EXPPOLY_ANT", Spec(
        body=((Src0 * C0 + C1) * Src0 + One) * Src0 + One,
        reference=lambda in0, s0, s1: ((in0 * s0 + s1) * in0 + 1) * in0 + 1,
    ))
    sq6 = mk("SQ6_ANT", Spec(
        body=sq(sq(sq(sq(sq(sq(Src0)))))),
        reference=lambda in0: in0 ** 64,
    ))
    _CACHE["dve"] = (exppoly, sq6)
    return exppoly, sq6


def _build_program():
    EXPPOLY, SQ6 = _register_dve_ops()
    nc = bacc.Bacc("TRN2", target_bir_lowering=False, debug=False)

    xt_d = nc.dram_tensor("xt", [E, S], F32R, kind="ExternalInput").ap()
    wq_d = nc.dram_tensor("wq", [E, G], F32R, kind="ExternalInput").ap()
    wk_d = nc.dram_tensor("wk", [E, G], F32R, kind="ExternalInput").ap()
    wv_d = nc.dram_tensor("wv", [E, G], F32R, kind="ExternalInput").ap()
    wo_d = nc.dram_tensor("wo", [G, E], F16, kind="ExternalInput").ap()
    bq_d = nc.dram_tensor("bq", [P, 4], F32, kind="ExternalInput").ap()
    bk_d = nc.dram_tensor("bk", [P, 4], F32, kind="ExternalInput").ap()
    bv_d = nc.dram_tensor("bv", [P, G], F32, kind="ExternalInput").ap()
    tri_d = nc.dram_tensor("tri", [P, P], F16, kind="ExternalInput").ap()
    one_d = nc.dram_tensor("one", [P, D], F16, kind="ExternalInput").ap()
    out_d = nc.dram_tensor("out", [S, E], F32, kind="ExternalOutput").ap()
    # scratch for the reciprocal-row broadcast bounce
    rc_d = nc.dram_tensor("rc_scratch", [HG, NQB, QB], F32, kind="Internal").ap()

    qt_sb = nc.alloc_sbuf_tensor("qt_sb", [P, 4, S], F16).ap()
    kt_sb = nc.alloc_sbuf_tensor("kt_sb", [P, 4, S], F16).ap()
    vx_sb = nc.alloc_sbuf_tensor("vx_sb", [P, NKT, HG, VW], F16).ap()
    tri_sb = nc.alloc_sbuf_tensor("tri_sb", [P, P], F16).ap()
    ones_sb = nc.alloc_sbuf_tensor("ones_sb", [P, D], F16).ap()
    bq_sb = nc.alloc_sbuf_tensor("bq_sb", [P, 4], F32).ap()
    bk_sb = nc.alloc_sbuf_tensor("bk_sb", [P, 4], F32).ap()
    bv_sb = nc.alloc_sbuf_tensor("bv_sb", [P, G], F32).ap()

    with tile.TileContext(nc) as tc:
        xt_r = xt_d.rearrange("(o p) s -> p o s", p=P)

        # ---- QKV projections (xT streamed per 512-token chunk) ----
        with (
            tc.tile_pool(name="w_pool", bufs=1) as wp,
            tc.tile_pool(name="xt_pool", bufs=2) as xp,
            tc.tile_pool(name="proj_ps", bufs=2, space="PSUM") as pp,
        ):
            wqp = wp.tile([P, ESUB, G], F32R, tag="wq", name="wqp")
            wkp = wp.tile([P, ESUB, G], F32R, tag="wk", name="wkp")
            wvp = wp.tile([P, ESUB, G], F32R, tag="wv", name="wvp")
            # first xT chunk loads ahead of the weights so the PE starts
            # sooner; wq fully precedes wk/wv so Q matmuls start first
            xtp0 = xp.tile([P, ESUB, QB], F32R, tag="xt", name="xtp")
            nc.sync.dma_start(xtp0[:], xt_r[:, :, 0:QB])
            for e in range(ESUB):
                nc.sync.dma_start(
                    wqp[:, e, :], wq_d.rearrange("(o p) f -> p o f", p=P)[:, e, :]
                )
            nc.sync.dma_start(bq_sb[:], bq_d[:])
            for e in range(ESUB):
                nc.sync.dma_start(
                    wkp[:, e, :], wk_d.rearrange("(o p) f -> p o f", p=P)[:, e, :]
                )
            nc.sync.dma_start(bk_sb[:], bk_d[:])
            for e in range(ESUB):
                nc.sync.dma_start(
                    wvp[:, e, :], wv_d.rearrange("(o p) f -> p o f", p=P)[:, e, :]
                )
            nc.sync.dma_start(bv_sb[:], bv_d[:])
            nc.sync.dma_start(tri_sb[:], tri_d[:])
            nc.sync.dma_start(ones_sb[:], one_d[:])
            for qb in range(NQB):
                if qb == 0:
                    xtp = xtp0
                else:
                    xtp = xp.tile([P, ESUB, QB], F32R, tag="xt", name="xtp")
                    nc.sync.dma_start(xtp[:], xt_r[:, :, qb * QB : (qb + 1) * QB])
                for cc in range(4):
                    q_ps = pp.tile([P, QB], F32, tag="q", name="q_ps")
                    for e in range(ESUB):
                        nc.tensor.matmul(
                            q_ps[:],
                            lhsT=wqp[:, e, cc * P : (cc + 1) * P],
                            rhs=xtp[:, e, :],
                            start=(e == 0),
                            stop=(e == ESUB - 1),
                        )
                    # fold bias and the 1/64 share of the softmax scale into Q
                    # (stored fp16; K carries the other 1/8)
                    nc.vector.tensor_scalar(
                        qt_sb[:, cc, qb * QB : (qb + 1) * QB],
                        q_ps[:],
                        bq_sb[:, cc : cc + 1],
                        1.0 / 64.0,
                        ADD,
                        MULT,
                    )
                    k_ps = pp.tile([P, QB], F32, tag="k", name="k_ps")
                    for e in range(ESUB):
                        nc.tensor.matmul(
                            k_ps[:],
                            lhsT=wkp[:, e, cc * P : (cc + 1) * P],
                            rhs=xtp[:, e, :],
                            start=(e == 0),
                            stop=(e == ESUB - 1),
                        )
                    nc.vector.tensor_scalar(
                        kt_sb[:, cc, qb * QB : (qb + 1) * QB],
                        k_ps[:],
                        bk_sb[:, cc : cc + 1],
                        1.0 / 8.0,
                        ADD,
                        MULT,
                    )
                for stl in range(4):
                    st = qb * 4 + stl
                    v_ps = pp.tile([P, G], F32, tag="v", name="v_ps")
                    for e in range(ESUB):
                        nc.tensor.matmul(
                            v_ps[:],
                            lhsT=xtp[:, e, stl * P : (stl + 1) * P],
                            rhs=wvp[:, e, :],
                            start=(e == 0),
                            stop=(e == ESUB - 1),
                        )
                    nc.vector.tensor_tensor(
                        vx_sb[:, st, :, 0:D],
                        v_ps.rearrange("p (h d) -> p h d", d=D),
                        bv_sb.rearrange("p (h d) -> p h d", d=D),
                        ADD,
                    )
                    nc.vector.tensor_copy(
                        vx_sb[:, st, :, D : D + 1],
                        ones_sb[:, 0:HG].rearrange("p (h u) -> p h u", u=1),
                    )

        # ---- attention + output projection ----
        with tc.tile_pool(name="at_pool", bufs=1) as atp:
            at_t = atp.tile([P, 4, S], F16, name="at_t")
            # wo lives in the same long-lived pool and loads during attention
            wop = atp.tile([P, 4, E], F16, name="wop")
            nc.sync.dma_start(wop[:], wo_d.rearrange("(o p) n -> p o n", p=P))
            with (
                tc.tile_pool(name="attn_ps", bufs=1, space="PSUM") as ap,
                tc.tile_pool(name="attn_sb", bufs=2) as sp,
            ):
                # heads run in even/odd pairs: the pair's score matmuls use
                # partition bases 0/64 (distinct PE row groups -> concurrent).
                # AV matmuls lag the scores by AV_LAG k-tiles so the PE never
                # waits on exp/mask; accumulators are per-(head, q-block) so
                # normalization runs mid-pass and frees PSUM slots early.
                AV_LAG = 4
                dbl_ctr = [0]

                def emit_outproj(st, n, drain):
                    o_ps = ap.tile([P, QB], F32, tag="a", name="o_ps", bufs=4)
                    for t in range(4):
                        nc.tensor.matmul(
                            o_ps[:],
                            lhsT=at_t[:, t, st * P : (st + 1) * P],
                            rhs=wop[:, t, n * QB : (n + 1) * QB],
                            start=(t == 0),
                            stop=(t == 3),
                        )
                    o_sb = sp.tile([P, QB], F32, tag="ost", name="o_sb", bufs=3)
                    if drain == "vector":
                        nc.vector.tensor_copy(o_sb[:], o_ps[:])
                    else:
                        nc.scalar.copy(o_sb[:], o_ps[:])
                    nc.sync.dma_start(
                        out_d[st * P : (st + 1) * P, n * QB : (n + 1) * QB],
                        o_sb[:],
                    )

                # output projection tiles st<8 interleave into the qb=2/3
                # rounds, st 8-11 into qb=3 (inputs ready block-granularity;
                # emitted only once their at_t inputs are certainly written,
                # so they never head-of-line-block the in-order PE queue)
                feed_a = [(st, n) for st in range(8) for n in range(2)]
                feed_b = [(st, n) for st in range(8, 12) for n in range(2)]

                norm_done = [0, 0, 0, 0]

                def normalize(ctx, h, qb):
                    a_t, heads, sub = ctx
                    norm_done[qb] += 1
                    hb = (h % 2) * D
                    a_ps = a_t[h]
                    # broadcast the RAW denominator row to 64 partitions via a
                    # DRAM bounce (a DRAM source AP may repeat along
                    # partitions, SBUF cannot), then take the reciprocal at
                    # partition base 0 with the fast custom-DVE op (~3e-6 rel
                    # err; it mishandles nonzero partition bases, hence this
                    # order)
                    dn = sp.tile([VW, QB], F32, tag="lg", name="dn", bufs=2)
                    nc.vector.tensor_copy(dn[D:VW, :], a_ps[D:VW, :])
                    nc.sync.dma_start(rc_d[h, qb : qb + 1, :], dn[D:VW, :])
                    db = sp.tile([D, QB], F32, tag="rs", name="db", bufs=2)
                    nc.sync.dma_start(
                        db[:], rc_d[h, qb : qb + 1, :].to_broadcast([D, QB])
                    )
                    rb_sb = sp.tile([D, QB], F32, tag="rbs", name="rb_sb", bufs=2)
                    nc.vector.reciprocal_approx_fast(rb_sb[:], db[:])
                    at_slice = at_t[hb : hb + D, sub, qb * QB : (qb + 1) * QB]
                    if hb == 0:
                        nc.vector.tensor_tensor(at_slice, a_ps[0:D, :], rb_sb[:], MULT)
                    else:
                        tmp = sp.tile([D, QB], F16, tag="tmp", name="tmp", bufs=2)
                        nc.vector.tensor_tensor(tmp[:], a_ps[0:D, :], rb_sb[:], MULT)
                        nc.sync.dma_start(at_slice, tmp[:])

                def av_main(ctx, pt, kt, qb):
                    # AV over the columns with no triangle-mask dependency:
                    # everything right of the diagonal 128-block (diag k-tile)
                    # or the whole live range (plain k-tile)
                    a_t, heads, sub = ctx
                    if kt // 4 == qb:
                        off = P * (kt % 4 + 1)
                    else:
                        off = 0
                    if off >= QB:
                        return
                    for h in heads:
                        nc.tensor.matmul(
                            a_t[h][0:VW, off:],
                            lhsT=vx_sb[:, kt, h, :],
                            rhs=pt[:, h % 2, off:],
                            start=(kt == 0),
                            stop=False,
                        )

                def av_tri(ctx, pt, kt, qb):
                    # the diagonal 128-column block, gated on the GpSimd
                    # triangle multiply; lags further so the in-order PE queue
                    # never stalls on it.  Carries start (kt==0, runs first
                    # into a fresh bank only for qb==0) and stop/normalize on
                    # the final k-tile.
                    a_t, heads, sub = ctx
                    m = kt % 4
                    last = kt == 4 * qb + 3
                    for h in heads:
                        nc.tensor.matmul(
                            a_t[h][0:VW, P * m : P * (m + 1)],
                            lhsT=vx_sb[:, kt, h, :],
                            rhs=pt[:, h % 2, P * m : P * (m + 1)],
                            start=False,
                            stop=last,
                        )
                    if last:
                        for h in heads:
                            normalize(ctx, h, qb)

                # one software pipeline across all (qb, pair) blocks: the AV/
                # normalize drain of each block interleaves with the next
                # block's score matmuls instead of stalling the in-order PE
                AV_LAG_TRI = 7
                pend_main = []   # (ctx, pt, kt, qb)
                pend_tri = []    # (ctx, pt, kt, qb)
                step = [0]

                def pump():
                    if len(pend_main) > AV_LAG:
                        av_main(*pend_main.pop(0))
                    if len(pend_tri) > AV_LAG_TRI:
                        av_tri(*pend_tri.pop(0))

                for qb in range(NQB):
                    rstep = 0
                    for pair in range(4):
                        heads = (2 * pair, 2 * pair + 1)
                        sub = pair
                        a_t = {
                            h: ap.tile([P, QB], F32, tag="a", name="a_ps", bufs=4)
                            for h in heads
                        }
                        ctx = (a_t, heads, sub)
                        for kt in range(4 * qb + 4):
                            s_t = ap.tile(
                                [P, 2, QB], F32, tag="s", name="s_ps", bufs=2
                            )
                            for h in heads:
                                hb = (h % 2) * D
                                nc.tensor.matmul(
                                    s_t[:, h % 2, :],
                                    lhsT=kt_sb[
                                        hb : hb + D, sub, kt * P : (kt + 1) * P
                                    ],
                                    rhs=qt_sb[
                                        hb : hb + D, sub, qb * QB : (qb + 1) * QB
                                    ],
                                    start=True,
                                    stop=True,
                                )
                            pt = sp.tile(
                                [P, 2, QB], F16, tag="pt", name="pt", bufs=10
                            )
                            diag = kt // 4 == qb
                            m = kt % 4 if diag else 0
                            # exp engine split: rotate a fixed share of tiles
                            # onto VectorE (two-op poly^64), rest on ScalarE
                            step[0] += 1
                            if step[0] % 5 == 2 and not diag:
                                tx = sp.tile(
                                    [P, 2, QB], F32, tag="tx", name="tx", bufs=2
                                )
                                nc.vector._custom_dve(
                                    EXPPOLY,
                                    out=tx[:, :, P * m :],
                                    in0=s_t[:, :, P * m :],
                                    s0=1.0 / 6.0,
                                    s1=0.5,
                                )
                                nc.vector._custom_dve(
                                    SQ6,
                                    out=pt[:, :, P * m :],
                                    in0=tx[:, :, P * m :],
                                )
                            else:
                                nc.scalar.activation(
                                    pt[:, :, P * m :], s_t[:, :, P * m :],
                                    Exp, scale=64.0,
                                )
                            if diag:
                                for h in heads:
                                    nc.gpsimd.tensor_tensor(
                                        pt[:, h % 2, P * m : P * (m + 1)],
                                        pt[:, h % 2, P * m : P * (m + 1)],
                                        tri_sb[:],
                                        MULT,
                                    )
                                pend_tri.append((ctx, pt, kt, qb))
                            pend_main.append((ctx, pt, kt, qb))
                            pump()
                            rstep += 1
                            if (
                                qb >= 2
                                and feed_a
                                and rstep % 3 == 0
                                and norm_done[feed_a[0][0] // 4] == 8
                            ):
                                emit_outproj(*feed_a.pop(0), drain="scalar")
                            elif (
                                qb == 3
                                and feed_b
                                and rstep % 4 == 2
                                and norm_done[feed_b[0][0] // 4] == 8
                            ):
                                emit_outproj(*feed_b.pop(0), drain="scalar")
                while pend_main or pend_tri:
                    if pend_main:
                        av_main(*pend_main.pop(0))
                    if pend_tri and (
                        not pend_main
                        or len(pend_tri) > AV_LAG_TRI - AV_LAG
                    ):
                        av_tri(*pend_tri.pop(0))
                for st_n in feed_a + feed_b:
                    emit_outproj(*st_n, drain="scalar")

            # ---- second-half output projection (first half ran inside the
            # attention loop; host adds the other group's partial + bo) ----
            with (
                tc.tile_pool(name="op_ps", bufs=2, space="PSUM") as op,
                tc.tile_pool(name="op_sb", bufs=3) as osp,
            ):
                for st in range(12, NST):
                    for n in range(2):
                        o_ps = op.tile([P, QB], F32, tag="o", name="o_ps")
                        for t in range(4):
                            nc.tensor.matmul(
                                o_ps[:],
                                lhsT=at_t[:, t, st * P : (st + 1) * P],
                                rhs=wop[:, t, n * QB : (n + 1) * QB],
                                start=(t == 0),
                                stop=(t == 3),
                            )
                        o_sb = osp.tile([P, QB], F32, tag="ost", name="o_sb")
                        nc.scalar.copy(o_sb[:], o_ps[:])
                        nc.sync.dma_start(
                            out_d[st * P : (st + 1) * P, n * QB : (n + 1) * QB],
                            o_sb[:],
                        )

    nc.compile()
    return nc


def _prep_inputs(x, Wqkv, bqkv, Wo, bo):
    x = np.asarray(x, np.float32)
    Wqkv = np.asarray(Wqkv, np.float32)
    bqkv = np.asarray(bqkv, np.float32)
    Wo = np.asarray(Wo, np.float32)

    # 128x128 inclusive lower-triangle-in-(q,k) == kl <= ql in [k, q] layout
    kl = np.arange(P)[:, None]
    tri = (kl <= np.arange(P)[None, :]).astype(np.float32)


    in_maps = []
    for c in range(8):
        b, g = divmod(c, 2)
        lo, hi = g * G, (g + 1) * G
        in_maps.append(
            {
                "xt": np.ascontiguousarray(x[b].T),
                "wq": np.ascontiguousarray(Wqkv[:, lo:hi]),
                "wk": np.ascontiguousarray(Wqkv[:, E + lo : E + hi]),
                "wv": np.ascontiguousarray(Wqkv[:, 2 * E + lo : 2 * E + hi]),
                "wo": np.ascontiguousarray(Wo[lo:hi, :]).astype(np.float16),
                "bq": np.ascontiguousarray(bqkv[lo:hi].reshape(4, P).T),
                "bk": np.ascontiguousarray(bqkv[E + lo : E + hi].reshape(4, P).T),
                "bv": np.tile(bqkv[2 * E + lo : 2 * E + hi][None, :], (P, 1)).astype(
                    np.float32
                ),
                "tri": tri.astype(np.float16),
                "one": np.ones((P, D), np.float16),
            }
        )
    return in_maps


def kernel(x, Wqkv, bqkv, Wo, bo, _trace=False):
    if "nc" not in _CACHE:
        _CACHE["nc"] = _build_program()
    nc = _CACHE["nc"]

    in_maps = _prep_inputs(x, Wqkv, bqkv, Wo, bo)
    res = run_bass_kernel_spmd(nc, in_maps, core_ids=list(range(8)), trace=_trace)
    _CACHE["last_result"] = res

    bo = np.asarray(bo, np.float32)
    out = np.empty((B, S, E), np.float32)
    for b in range(B):
        out[b] = res.results[2 * b]["out"] + res.results[2 * b + 1]["out"] + bo
    return out
